# revision 18
# baseline (speedup 1.0000x reference)
"""DiT attention block on 8 Trainium2 NeuronCores.

Reference computation (fp32):
    qkv = x @ Wqkv + b            (b=2, n=2048, din=1024, 3*1024)
    q, k = RMSNorm_full_dim(q|k) * scale  (norm over all 1024 channels)
    RoPE (rotary_dim=64) per 64-dim head, 16 heads
    attn = softmax(q k^T / 8) v ;  out = attn @ Wout + bout

Sharding: 8 cores = 2 batches x 4 head-groups (4 heads / 256 features each).
Two SPMD launches:
  L1: per-core slice of the qkv projection (transposed layout) + partial
      sum-of-squares for the full-dim RMSNorm (host combines: tiny rsqrt).
  L2: norm+rope (DVE), S^T = khat^T qhat (PE, row-tiled head pairs),
      exp (ACT, straight from PSUM), O^T|den = [V|1]^T P (PE, M=65),
      normalize via DVE reciprocal + gpsimd partition_broadcast,
      out-projection partial products.
Host: slices/transposes inputs, rsqrt between launches, sums the 4 partial
projections per batch and adds the (host-folded) v-bias/out-bias term.

All matmuls run as float32r (TF32-like, ~1e-4 rel err, full PE rate).
Column-tiled (tile_position[1] != 0) matmuls are illegal for fp32r, so the
softmax denominator rides along as a 65th column of V instead.
"""

import os
import sys

for _p in ("/opt/trn_rl_repo", "/root/.axon_site/_ro/trn_rl_repo"):
    if os.path.isdir(_p) and _p not in sys.path:
        sys.path.append(_p)

import numpy as np

import concourse.bass as bass  # noqa: E402,F401
import concourse.mybir as mybir  # noqa: E402
import concourse.tile as tile  # noqa: E402
from concourse import bacc  # noqa: E402
from concourse.bass_utils import run_bass_kernel_spmd  # noqa: E402

FP32 = mybir.dt.float32
FP32R = mybir.dt.float32r
AF = mybir.ActivationFunctionType

B = 2
N = 2048
DIN = 1024
DQ = 1024
H = 16
DH = 64
NCORES = 8
NGROUP = 4          # head-groups per batch
GF = 256            # features per core (4 heads)
P = 128
EPS = 1e-6
ROPE_BASE = 10000.0

LAST_EXEC_NS = {}   # filled when KERNEL_TRACE=1
LAST_RESULTS = {}   # BassKernelResults per launch when KERNEL_TRACE=1

_cache = {}


# ----------------------------------------------------------------- launch 1

def _build_l1():
    nc = bacc.Bacc("TRN2", target_bir_lowering=False, debug=False,
                   num_devices=NCORES)
    xT = nc.dram_tensor("xT", [DIN, N], FP32R, kind="ExternalInput")
    wcat = nc.dram_tensor("wcat", [DIN, 3 * GF], FP32R, kind="ExternalInput")
    bqk = nc.dram_tensor("bqk", [P, 4], FP32, kind="ExternalInput")
    invs = nc.dram_tensor("invs", [P, P], FP32R, kind="ExternalInput")
    qT_o = nc.dram_tensor("qT", [GF, N], FP32, kind="ExternalOutput")
    kT_o = nc.dram_tensor("kT", [GF, N], FP32, kind="ExternalOutput")
    v_o = nc.dram_tensor("v", [N, GF], FP32, kind="ExternalOutput")
    ssq_o = nc.dram_tensor("ssq", [2, N], FP32, kind="ExternalOutput")

    KT = DIN // P  # 8 contraction tiles
    NB = N // 512  # 4 column blocks

    with tile.TileContext(nc) as tc:
        with (
            tc.tile_pool(name="xw", bufs=1) as xw,
            tc.tile_pool(name="io", bufs=2) as io,
            tc.tile_pool(name="sqp", bufs=2) as sqp,
            tc.tile_pool(name="stgp", bufs=4) as stgp,
            tc.tile_pool(name="ps", bufs=2, space="PSUM") as ps,
            tc.tile_pool(name="pssq", bufs=4, space="PSUM") as pssq,
        ):
            xt = []
            wt = []
            for kt in range(KT):
                t = xw.tile([P, N], FP32R, tag=f"xt{kt}")
                nc.sync.dma_start(t[:], xT[kt * P:(kt + 1) * P, :])
                xt.append(t)
                w = xw.tile([P, 3 * GF], FP32R, tag=f"wt{kt}")
                nc.sync.dma_start(w[:], wcat[kt * P:(kt + 1) * P, :])
                wt.append(w)
            bias = xw.tile([P, 4], FP32, tag="bias")
            nc.sync.dma_start(bias[:], bqk[:, :])
            winv = xw.tile([P, P], FP32R, tag="winv")
            nc.sync.dma_start(winv[:], invs[:, :])

            # q', k' in transposed layout, with bias, + partial ssq
            for t_idx, (col0, out_dram) in enumerate(((0, qT_o), (GF, kT_o))):
                bigs = []
                for mt in range(GF // P):
                    big = io.tile([P, N], FP32, tag="qk")
                    for nb in range(NB):
                        acc = ps.tile([P, 512], FP32, tag="acc")
                        for kt in range(KT):
                            nc.tensor.matmul(
                                acc[:],
                                wt[kt][:, col0 + mt * P: col0 + (mt + 1) * P],
                                xt[kt][:, nb * 512:(nb + 1) * 512],
                                start=(kt == 0),
                                stop=(kt == KT - 1),
                            )
                        nc.scalar.activation(
                            big[:, nb * 512:(nb + 1) * 512], acc[:],
                            AF.Identity,
                            bias=bias[:, 2 * t_idx + mt: 2 * t_idx + mt + 1],
                        )
                    nc.sync.dma_start(out_dram[mt * P:(mt + 1) * P, :], big[:])
                    bigs.append(big)

                # partial weighted sum-of-squares for this tensor:
                # 32 identical output rows (all-equal lhsT columns), M=32 at
                # tile (0,0) -- col-tiled fp32r matmuls are illegal.
                for nb in range(NB):
                    sp = pssq.tile([32, 512], FP32, tag="ssq",
                                   name=f"ssq{t_idx}_{nb}")
                    for mt in range(GF // P):
                        sq = sqp.tile([P, 512], FP32R, tag="sq")
                        nc.vector.tensor_tensor(
                            sq[:],
                            bigs[mt][:, nb * 512:(nb + 1) * 512],
                            bigs[mt][:, nb * 512:(nb + 1) * 512],
                            mybir.AluOpType.mult)
                        nc.tensor.matmul(
                            sp[:],
                            winv[:, 32 * (2 * t_idx + mt):
                                 32 * (2 * t_idx + mt + 1)],
                            sq[:],
                            start=(mt == 0),
                            stop=(mt == GF // P - 1),
                        )
                    stg = stgp.tile([1, 512], FP32, tag="stg",
                                    name=f"stg{t_idx}_{nb}")
                    nc.vector.tensor_copy(stg[:], sp[0:1, :])
                    nc.sync.dma_start(
                        ssq_o[t_idx:t_idx + 1, nb * 512:(nb + 1) * 512],
                        stg[:])

            # v in natural (token, feature) layout, no bias
            for tt in range(N // P):
                acc = ps.tile([P, GF], FP32, tag="vacc")
                for kt in range(KT):
                    nc.tensor.matmul(
                        acc[:],
                        xt[kt][:, tt * P:(tt + 1) * P],
                        wt[kt][:, 2 * GF:3 * GF],
                        start=(kt == 0),
                        stop=(kt == KT - 1),
                    )
                vsb = io.tile([P, GF], FP32, tag="v")
                nc.vector.tensor_copy(vsb[:], acc[:])
                nc.sync.dma_start(v_o[tt * P:(tt + 1) * P, :], vsb[:])

    nc.compile()
    return nc


# ----------------------------------------------------------------- launch 2

def _build_l2():
    nc = bacc.Bacc("TRN2", target_bir_lowering=False, debug=False,
                   num_devices=NCORES)
    qT = nc.dram_tensor("qT", [GF, N], FP32, kind="ExternalInput")
    kT = nc.dram_tensor("kT", [GF, N], FP32, kind="ExternalInput")
    # v with a ones-column appended per head: [v_h (64) | 1] x 4 heads
    v_i = nc.dram_tensor("v", [N, 4 * 65], FP32R, kind="ExternalInput")
    cosq_i = nc.dram_tensor("cosq", [P, N], FP32, kind="ExternalInput")
    sinq_i = nc.dram_tensor("sinq", [P, N], FP32, kind="ExternalInput")
    cosk_i = nc.dram_tensor("cosk", [P, N], FP32, kind="ExternalInput")
    sink_i = nc.dram_tensor("sink", [P, N], FP32, kind="ExternalInput")
    wout_i = nc.dram_tensor("wout", [GF, DIN], FP32R, kind="ExternalInput")
    part_o = nc.dram_tensor("part", [2, N, DIN], FP32, kind="ExternalOutput")

    IBW = 512        # query-block width
    NIB = N // IBW   # 4 query blocks
    NJT = N // P     # 16 key tiles

    with tile.TileContext(nc) as tc:
        with (
            tc.tile_pool(name="cst", bufs=1) as cst,
            tc.tile_pool(name="scr", bufs=2) as scr,
            tc.tile_pool(name="hat", bufs=1) as hatp,
            tc.tile_pool(name="ptp", bufs=3) as ptp,
            tc.tile_pool(name="obig", bufs=1) as obigp,
            tc.tile_pool(name="onrm", bufs=2) as onrm,
            tc.tile_pool(name="outp", bufs=2) as outp,
            tc.tile_pool(name="tiny", bufs=2) as tiny,
            tc.tile_pool(name="psS", bufs=2, space="PSUM") as psS,
            tc.tile_pool(name="psO", bufs=4, space="PSUM") as psO,
        ):
            # ---- phase A: normalize + rope -> qhat/khat (fp32r) ----
            # The RMSNorm factor is folded into per-tensor cos/sin tables on
            # the host, so each tile chain is only 3 DVE passes.  Pair-0
            # tiles (k0, q0) first so pair-0 attention starts early.
            tabs = {}
            for name, ci, si in (("k", cosk_i, sink_i), ("q", cosq_i, sinq_i)):
                cr = cst.tile([P, N], FP32, tag=f"cos_{name}")
                nc.sync.dma_start(cr[:], ci[:, :])
                sr = cst.tile([P, N], FP32, tag=f"sin_{name}")
                nc.sync.dma_start(sr[:], si[:, :])
                tabs[name] = (cr, sr)
            hats = {}
            for name, src_dram, mt in (("k", kT, 0), ("q", qT, 0),
                                       ("k", kT, 1), ("q", qT, 1)):
                cr, sr = tabs[name]
                t1 = scr.tile([P, N], FP32, tag="t1")
                nc.sync.dma_start(t1[:], src_dram[mt * P:(mt + 1) * P, :])
                sh = scr.tile([P, N], FP32, tag="sh")
                for blk in range(4):
                    srcb = blk ^ 1
                    nc.sync.dma_start(sh[blk * 32:(blk + 1) * 32, :],
                                      t1[srcb * 32:(srcb + 1) * 32, :])
                t2 = scr.tile([P, N], FP32, tag="t2")
                nc.vector.tensor_mul(t2[:], t1[:], cr[:])
                nc.vector.tensor_mul(sh[:], sh[:], sr[:])
                hat = hatp.tile([P, N], FP32R, tag=f"hat_{name}{mt}")
                nc.vector.tensor_add(hat[:], t2[:], sh[:])
                hats[(name, mt)] = hat

            vt = []
            for jt in range(NJT):
                t = cst.tile([P, 4 * 65], FP32R, tag=f"v{jt}")
                nc.sync.dma_start(t[:], v_i[jt * P:(jt + 1) * P, :])
                vt.append(t)
            wout = []
            for kt in range(2):
                w = cst.tile([P, DIN], FP32R, tag=f"wo{kt}")
                nc.sync.dma_start(w[:], wout_i[kt * P:(kt + 1) * P, :])
                wout.append(w)

            # ---- phase B: attention ----
            # obig[pair]: normalized O^T for heads (2*pair, 2*pair+1).
            # Pair-outer loop: each head's S slice owns a full PSUM bank
            # (two start=True groups must never share a bank), and the
            # two-bank S tile double-buffers against the single exp call.
            obig = [obigp.tile([P, N], FP32R, tag=f"obig{pr}", name=f"ob{pr}")
                    for pr in range(2)]
            for pr in range(2):
                for ib in range(NIB):
                    o_ps = [psO.tile([65, IBW], FP32, tag="O",
                                     name=f"o{pr}_{ib}_{s}") for s in range(2)]
                    for jt in range(NJT):
                        s_ps = psS.tile([P, 2 * IBW], FP32, tag="S")
                        for sub in range(2):
                            nc.tensor.matmul(
                                s_ps[:, sub * IBW:(sub + 1) * IBW],
                                hats[("k", pr)][sub * 64:(sub + 1) * 64,
                                                jt * P:(jt + 1) * P],
                                hats[("q", pr)][sub * 64:(sub + 1) * 64,
                                                ib * IBW:(ib + 1) * IBW],
                                start=True, stop=True,
                                tile_position=(64 * sub, 0),
                            )
                        p_sb = ptp.tile([P, 2 * IBW], FP32R, tag="P")
                        nc.scalar.activation(p_sb[:, :], s_ps[:, :],
                                             AF.Exp, scale=0.125)
                        for sub in range(2):
                            h = 2 * pr + sub
                            nc.tensor.matmul(
                                o_ps[sub][:, :],
                                vt[jt][:, h * 65:(h + 1) * 65],
                                p_sb[:, sub * IBW:(sub + 1) * IBW],
                                start=(jt == 0), stop=(jt == NJT - 1),
                            )
                    # normalize: row 64 of each o_ps is the denominator
                    for sub in range(2):
                        rrd = tiny.tile([65, IBW], FP32, tag="rrd")
                        nc.vector.tensor_copy(rrd[64:65, :],
                                              o_ps[sub][64:65, :])
                        nc.vector.reciprocal(rrd[64:65, :], rrd[64:65, :])
                        # partition_broadcast reads the tile's partition 0,
                        # so stage the reciprocal row down via DMA
                        rr0 = tiny.tile([1, IBW], FP32, tag="rr0")
                        nc.sync.dma_start(rr0[:, :], rrd[64:65, :])
                        bc = tiny.tile([64, IBW], FP32, tag="bc")
                        nc.gpsimd.partition_broadcast(bc[:, :], rr0[:, :])
                        if sub == 0:
                            nc.vector.tensor_mul(
                                obig[pr][0:64, ib * IBW:(ib + 1) * IBW],
                                o_ps[sub][0:64, :], bc[:, :])
                        else:
                            onr = onrm.tile([64, IBW], FP32R, tag="onr")
                            nc.vector.tensor_mul(onr[:, :],
                                                 o_ps[sub][0:64, :], bc[:, :])
                            nc.sync.dma_start(
                                obig[pr][64:128, ib * IBW:(ib + 1) * IBW],
                                onr[:, :])

                # ---- per-pair out-projection partials, emitted right
                # after this pair's attention so the scheduler interleaves
                # pair-0 projection into pair-1's (ACT-bound) attention;
                # the host adds the two partial outputs.
                for tt in range(N // P):
                    pss = [psO.tile([P, 512], FP32, tag="O",
                                    name=f"pj{pr}_{tt}_{hf}")
                           for hf in range(2)]
                    for half in range(2):
                        nc.tensor.matmul(
                            pss[half][:],
                            obig[pr][:, tt * P:(tt + 1) * P],
                            wout[pr][:, half * 512:(half + 1) * 512],
                            start=True, stop=True,
                        )
                    osb = outp.tile([P, DIN], FP32, tag="osb")
                    if pr == 0:
                        nc.vector.tensor_copy(osb[:, 0:512], pss[0][:])
                        nc.vector.tensor_copy(osb[:, 512:1024], pss[1][:])
                    else:
                        # pair-1 projection drains after the last exp; ACT
                        # is idle there, so give it the PSUM evacuation
                        nc.scalar.activation(osb[:, 0:512], pss[0][:],
                                             AF.Copy)
                        nc.scalar.activation(osb[:, 512:1024], pss[1][:],
                                             AF.Copy)
                    nc.sync.dma_start(part_o[pr, tt * P:(tt + 1) * P, :],
                                      osb[:])

    nc.compile()
    return nc


# ------------------------------------------------------------------- driver

def _rope_tables():
    half = DH // 2
    inv_freq = 1.0 / (ROPE_BASE ** (np.arange(half, dtype=np.float64) * 2.0
                                    / DH))
    freqs = np.arange(N, dtype=np.float64)[:, None] * inv_freq[None, :]
    cos = np.cos(freqs).T          # (32, N)
    sin = np.sin(freqs).T
    cos64 = np.concatenate([cos, cos], 0)            # (64, N)
    sin64 = np.concatenate([-sin, sin], 0)           # signed for rotate_half
    cos_t = np.ascontiguousarray(
        np.concatenate([cos64, cos64], 0).astype(np.float32))  # (128, N)
    sin_t = np.ascontiguousarray(
        np.concatenate([sin64, sin64], 0).astype(np.float32))
    return cos_t, sin_t


def kernel(input, w_qkv, b_qkv, q_scale, k_scale, w_out, b_out):
    trace = bool(os.environ.get("KERNEL_TRACE"))
    if "l1" not in _cache:
        _cache["l1"] = _build_l1()
    if "l2" not in _cache:
        _cache["l2"] = _build_l2()

    x = np.asarray(input, dtype=np.float32)
    w_qkv = np.asarray(w_qkv, dtype=np.float32)
    b_qkv = np.asarray(b_qkv, dtype=np.float32)
    qs = np.asarray(q_scale, dtype=np.float32)
    ks = np.asarray(k_scale, dtype=np.float32)
    w_out = np.asarray(w_out, dtype=np.float32)
    b_out = np.asarray(b_out, dtype=np.float32)

    wq = w_qkv[:, :DQ] * qs[None, :]
    wk = w_qkv[:, DQ:2 * DQ] * ks[None, :]
    wv = w_qkv[:, 2 * DQ:]
    bq = b_qkv[:DQ] * qs
    bk = b_qkv[DQ:2 * DQ] * ks
    bv = b_qkv[2 * DQ:]

    xT = [np.ascontiguousarray(x[b].T) for b in range(B)]

    def col4(vec256_a, vec256_b):
        # -> (128, 4): [a_mt0 | a_mt1 | b_mt0 | b_mt1]
        return np.ascontiguousarray(np.stack(
            [vec256_a[:P], vec256_a[P:], vec256_b[:P], vec256_b[P:]],
            axis=1).astype(np.float32))

    in1 = []
    for c in range(NCORES):
        b, g = divmod(c, NGROUP)
        sl = slice(g * GF, (g + 1) * GF)
        wcat = np.ascontiguousarray(
            np.concatenate([wq[:, sl], wk[:, sl], wv[:, sl]], axis=1))
        in1.append({
            "xT": xT[b],
            "wcat": wcat,
            "bqk": col4(bq[sl], bk[sl]),
            "invs": np.ascontiguousarray(np.repeat(
                col4(1.0 / np.square(qs[sl]), 1.0 / np.square(ks[sl])),
                32, axis=1)),
        })

    r1 = run_bass_kernel_spmd(_cache["l1"], in1,
                              core_ids=list(range(NCORES)), trace=trace)
    if trace:
        LAST_EXEC_NS["l1"] = r1.exec_time_ns
        LAST_RESULTS["l1"] = r1

    # host: combine partial ssq -> rsqrt factors folded into rope tables
    cos_t, sin_t = _rope_tables()
    tabs = {}
    for b in range(B):
        sq_q = np.zeros(N, np.float64)
        sq_k = np.zeros(N, np.float64)
        for g in range(NGROUP):
            ssq = r1.results[NGROUP * b + g]["ssq"].astype(np.float64)
            sq_q += ssq[0]
            sq_k += ssq[1]
        r_q = (1.0 / np.sqrt(sq_q / DQ + EPS)).astype(np.float32)
        r_k = (1.0 / np.sqrt(sq_k / DQ + EPS)).astype(np.float32)
        tabs[b] = {
            "cosq": np.ascontiguousarray(cos_t * r_q[None, :]),
            "sinq": np.ascontiguousarray(sin_t * r_q[None, :]),
            "cosk": np.ascontiguousarray(cos_t * r_k[None, :]),
            "sink": np.ascontiguousarray(sin_t * r_k[None, :]),
        }

    in2 = []
    for c in range(NCORES):
        b, g = divmod(c, NGROUP)
        sl = slice(g * GF, (g + 1) * GF)
        v = r1.results[c]["v"]                       # (N, 256)
        v65 = np.ones((N, 4 * 65), np.float32)
        for h in range(4):
            v65[:, h * 65:h * 65 + 64] = v[:, h * 64:(h + 1) * 64]
        in2.append({
            "qT": r1.results[c]["qT"],
            "kT": r1.results[c]["kT"],
            "v": np.ascontiguousarray(v65),
            "wout": np.ascontiguousarray(w_out[sl, :]),
            **tabs[b],
        })

    r2 = run_bass_kernel_spmd(_cache["l2"], in2,
                              core_ids=list(range(NCORES)), trace=trace)
    if trace:
        LAST_EXEC_NS["l2"] = r2.exec_time_ns
        LAST_RESULTS["l2"] = r2

    base = (bv.astype(np.float64) @ w_out.astype(np.float64)
            + b_out.astype(np.float64))
    out = np.zeros((B, N, DIN), np.float32)
    for b in range(B):
        acc = np.zeros((N, DIN), np.float64)
        for g in range(NGROUP):
            p = r2.results[NGROUP * b + g]["part"].astype(np.float64)
            acc += p[0]
            acc += p[1]
        out[b] = (acc + base[None, :]).astype(np.float32)
    return out


# revision 20
# speedup vs baseline: 1.0030x; 1.0030x over previous
"""DiT attention block on 8 Trainium2 NeuronCores.

Reference computation (fp32):
    qkv = x @ Wqkv + b            (b=2, n=2048, din=1024, 3*1024)
    q, k = RMSNorm_full_dim(q|k) * scale  (norm over all 1024 channels)
    RoPE (rotary_dim=64) per 64-dim head, 16 heads
    attn = softmax(q k^T / 8) v ;  out = attn @ Wout + bout

Sharding: 8 cores = 2 batches x 4 head-groups (4 heads / 256 features each).
Two SPMD launches:
  L1: per-core slice of the qkv projection (transposed layout) + partial
      sum-of-squares for the full-dim RMSNorm (host combines: tiny rsqrt).
  L2: norm+rope (DVE), S^T = khat^T qhat (PE, row-tiled head pairs),
      exp (ACT, straight from PSUM), O^T|den = [V|1]^T P (PE, M=65),
      normalize via DVE reciprocal + gpsimd partition_broadcast,
      out-projection partial products.
Host: slices/transposes inputs, rsqrt between launches, sums the 4 partial
projections per batch and adds the (host-folded) v-bias/out-bias term.

All matmuls run as float32r (TF32-like, ~1e-4 rel err, full PE rate).
Column-tiled (tile_position[1] != 0) matmuls are illegal for fp32r, so the
softmax denominator rides along as a 65th column of V instead.
"""

import os
import sys

for _p in ("/opt/trn_rl_repo", "/root/.axon_site/_ro/trn_rl_repo"):
    if os.path.isdir(_p) and _p not in sys.path:
        sys.path.append(_p)

import numpy as np

import concourse.bass as bass  # noqa: E402,F401
import concourse.mybir as mybir  # noqa: E402
import concourse.tile as tile  # noqa: E402
from concourse import bacc  # noqa: E402
from concourse.bass_utils import run_bass_kernel_spmd  # noqa: E402

FP32 = mybir.dt.float32
FP32R = mybir.dt.float32r
AF = mybir.ActivationFunctionType

B = 2
N = 2048
DIN = 1024
DQ = 1024
H = 16
DH = 64
NCORES = 8
NGROUP = 4          # head-groups per batch
GF = 256            # features per core (4 heads)
P = 128
EPS = 1e-6
ROPE_BASE = 10000.0

LAST_EXEC_NS = {}   # filled when KERNEL_TRACE=1
LAST_RESULTS = {}   # BassKernelResults per launch when KERNEL_TRACE=1

_cache = {}


# ----------------------------------------------------------------- launch 1

def _build_l1():
    nc = bacc.Bacc("TRN2", target_bir_lowering=False, debug=False,
                   num_devices=NCORES)
    xT = nc.dram_tensor("xT", [DIN, N], FP32R, kind="ExternalInput")
    wcat = nc.dram_tensor("wcat", [DIN, 3 * GF], FP32R, kind="ExternalInput")
    bqk = nc.dram_tensor("bqk", [P, 4], FP32, kind="ExternalInput")
    invs = nc.dram_tensor("invs", [P, P], FP32R, kind="ExternalInput")
    qT_o = nc.dram_tensor("qT", [GF, N], FP32, kind="ExternalOutput")
    kT_o = nc.dram_tensor("kT", [GF, N], FP32, kind="ExternalOutput")
    v_o = nc.dram_tensor("v", [N, GF], FP32, kind="ExternalOutput")
    ssq_o = nc.dram_tensor("ssq", [2, N], FP32, kind="ExternalOutput")

    KT = DIN // P  # 8 contraction tiles
    NB = N // 512  # 4 column blocks

    with tile.TileContext(nc) as tc:
        with (
            tc.tile_pool(name="xw", bufs=1) as xw,
            tc.tile_pool(name="io", bufs=2) as io,
            tc.tile_pool(name="sqp", bufs=2) as sqp,
            tc.tile_pool(name="stgp", bufs=4) as stgp,
            tc.tile_pool(name="ps", bufs=2, space="PSUM") as ps,
            tc.tile_pool(name="pssq", bufs=4, space="PSUM") as pssq,
        ):
            xt = []
            wt = []
            for kt in range(KT):
                t = xw.tile([P, N], FP32R, tag=f"xt{kt}")
                nc.sync.dma_start(t[:], xT[kt * P:(kt + 1) * P, :])
                xt.append(t)
                w = xw.tile([P, 3 * GF], FP32R, tag=f"wt{kt}")
                nc.sync.dma_start(w[:], wcat[kt * P:(kt + 1) * P, :])
                wt.append(w)
            bias = xw.tile([P, 4], FP32, tag="bias")
            nc.sync.dma_start(bias[:], bqk[:, :])
            winv = xw.tile([P, P], FP32R, tag="winv")
            nc.sync.dma_start(winv[:], invs[:, :])

            # q', k' in transposed layout, with bias, + partial ssq
            for t_idx, (col0, out_dram) in enumerate(((0, qT_o), (GF, kT_o))):
                bigs = []
                for mt in range(GF // P):
                    big = io.tile([P, N], FP32, tag="qk")
                    for nb in range(NB):
                        acc = ps.tile([P, 512], FP32, tag="acc")
                        for kt in range(KT):
                            nc.tensor.matmul(
                                acc[:],
                                wt[kt][:, col0 + mt * P: col0 + (mt + 1) * P],
                                xt[kt][:, nb * 512:(nb + 1) * 512],
                                start=(kt == 0),
                                stop=(kt == KT - 1),
                            )
                        nc.scalar.activation(
                            big[:, nb * 512:(nb + 1) * 512], acc[:],
                            AF.Identity,
                            bias=bias[:, 2 * t_idx + mt: 2 * t_idx + mt + 1],
                        )
                    nc.sync.dma_start(out_dram[mt * P:(mt + 1) * P, :], big[:])
                    bigs.append(big)

                # partial weighted sum-of-squares for this tensor:
                # 32 identical output rows (all-equal lhsT columns), M=32 at
                # tile (0,0) -- col-tiled fp32r matmuls are illegal.
                for nb in range(NB):
                    sp = pssq.tile([32, 512], FP32, tag="ssq",
                                   name=f"ssq{t_idx}_{nb}")
                    for mt in range(GF // P):
                        sq = sqp.tile([P, 512], FP32R, tag="sq")
                        nc.vector.tensor_tensor(
                            sq[:],
                            bigs[mt][:, nb * 512:(nb + 1) * 512],
                            bigs[mt][:, nb * 512:(nb + 1) * 512],
                            mybir.AluOpType.mult)
                        nc.tensor.matmul(
                            sp[:],
                            winv[:, 32 * (2 * t_idx + mt):
                                 32 * (2 * t_idx + mt + 1)],
                            sq[:],
                            start=(mt == 0),
                            stop=(mt == GF // P - 1),
                        )
                    stg = stgp.tile([1, 512], FP32, tag="stg",
                                    name=f"stg{t_idx}_{nb}")
                    nc.vector.tensor_copy(stg[:], sp[0:1, :])
                    nc.sync.dma_start(
                        ssq_o[t_idx:t_idx + 1, nb * 512:(nb + 1) * 512],
                        stg[:])

            # v in natural (token, feature) layout, no bias
            for tt in range(N // P):
                acc = ps.tile([P, GF], FP32, tag="vacc")
                for kt in range(KT):
                    nc.tensor.matmul(
                        acc[:],
                        xt[kt][:, tt * P:(tt + 1) * P],
                        wt[kt][:, 2 * GF:3 * GF],
                        start=(kt == 0),
                        stop=(kt == KT - 1),
                    )
                vsb = io.tile([P, GF], FP32, tag="v")
                nc.vector.tensor_copy(vsb[:], acc[:])
                nc.sync.dma_start(v_o[tt * P:(tt + 1) * P, :], vsb[:])

    nc.compile()
    return nc


# ----------------------------------------------------------------- launch 2

def _build_l2():
    nc = bacc.Bacc("TRN2", target_bir_lowering=False, debug=False,
                   num_devices=NCORES)
    qT = nc.dram_tensor("qT", [GF, N], FP32, kind="ExternalInput")
    kT = nc.dram_tensor("kT", [GF, N], FP32, kind="ExternalInput")
    # v with a ones-column appended per head: [v_h (64) | 1] x 4 heads
    v_i = nc.dram_tensor("v", [N, 4 * 65], FP32R, kind="ExternalInput")
    cosq_i = nc.dram_tensor("cosq", [P, N], FP32, kind="ExternalInput")
    sinq_i = nc.dram_tensor("sinq", [P, N], FP32, kind="ExternalInput")
    cosk_i = nc.dram_tensor("cosk", [P, N], FP32, kind="ExternalInput")
    sink_i = nc.dram_tensor("sink", [P, N], FP32, kind="ExternalInput")
    wout_i = nc.dram_tensor("wout", [GF, DIN], FP32R, kind="ExternalInput")
    part_o = nc.dram_tensor("part", [2, N, DIN], FP32, kind="ExternalOutput")

    IBW = 512        # query-block width
    NIB = N // IBW   # 4 query blocks
    NJT = N // P     # 16 key tiles

    with tile.TileContext(nc) as tc:
        with (
            tc.tile_pool(name="cst", bufs=1) as cst,
            tc.tile_pool(name="scr", bufs=2) as scr,
            tc.tile_pool(name="hat", bufs=1) as hatp,
            tc.tile_pool(name="ptp", bufs=3) as ptp,
            tc.tile_pool(name="obig", bufs=1) as obigp,
            tc.tile_pool(name="onrm", bufs=2) as onrm,
            tc.tile_pool(name="outp", bufs=2) as outp,
            tc.tile_pool(name="tiny", bufs=2) as tiny,
            tc.tile_pool(name="psS", bufs=2, space="PSUM") as psS,
            tc.tile_pool(name="psO", bufs=4, space="PSUM") as psO,
        ):
            # ---- phase A: normalize + rope -> qhat/khat (fp32r) ----
            # The RMSNorm factor is folded into per-tensor cos/sin tables on
            # the host, so each tile chain is only 3 DVE passes.  Pair-0
            # tiles (k0, q0) first so pair-0 attention starts early.
            tabs = {}
            for name, ci, si in (("k", cosk_i, sink_i), ("q", cosq_i, sinq_i)):
                cr = cst.tile([P, N], FP32, tag=f"cos_{name}")
                nc.sync.dma_start(cr[:], ci[:, :])
                sr = cst.tile([P, N], FP32, tag=f"sin_{name}")
                nc.sync.dma_start(sr[:], si[:, :])
                tabs[name] = (cr, sr)
            hats = {}
            for name, src_dram, mt in (("k", kT, 0), ("q", qT, 0),
                                       ("k", kT, 1), ("q", qT, 1)):
                cr, sr = tabs[name]
                t1 = scr.tile([P, N], FP32, tag="t1")
                nc.sync.dma_start(t1[:], src_dram[mt * P:(mt + 1) * P, :])
                sh = scr.tile([P, N], FP32, tag="sh")
                for blk in range(4):
                    srcb = blk ^ 1
                    nc.sync.dma_start(sh[blk * 32:(blk + 1) * 32, :],
                                      t1[srcb * 32:(srcb + 1) * 32, :])
                t2 = scr.tile([P, N], FP32, tag="t2")
                nc.vector.tensor_mul(t2[:], t1[:], cr[:])
                nc.vector.tensor_mul(sh[:], sh[:], sr[:])
                hat = hatp.tile([P, N], FP32R, tag=f"hat_{name}{mt}")
                nc.vector.tensor_add(hat[:], t2[:], sh[:])
                hats[(name, mt)] = hat

            vt = []
            for jt in range(NJT):
                t = cst.tile([P, 4 * 65], FP32R, tag=f"v{jt}")
                nc.sync.dma_start(t[:], v_i[jt * P:(jt + 1) * P, :])
                vt.append(t)
            wout = []
            for kt in range(2):
                w = cst.tile([P, DIN], FP32R, tag=f"wo{kt}")
                nc.sync.dma_start(w[:], wout_i[kt * P:(kt + 1) * P, :])
                wout.append(w)

            def emit_proj(pr, tt):
                # one token-tile of the out-projection partial for pair pr;
                # PE is in-order, so these are interleaved into pair-1's
                # (ACT-bound) attention stream. Host adds the two partials.
                pss = [psO.tile([P, 512], FP32, tag="O",
                                name=f"pj{pr}_{tt}_{hf}")
                       for hf in range(2)]
                for half in range(2):
                    nc.tensor.matmul(
                        pss[half][:],
                        obig[pr][:, tt * P:(tt + 1) * P],
                        wout[pr][:, half * 512:(half + 1) * 512],
                        start=True, stop=True,
                    )
                osb = outp.tile([P, DIN], FP32, tag="osb")
                nc.vector.tensor_copy(osb[:, 0:512], pss[0][:])
                nc.vector.tensor_copy(osb[:, 512:1024], pss[1][:])
                nc.sync.dma_start(part_o[pr, tt * P:(tt + 1) * P, :], osb[:])

            # ---- phase B: attention ----
            # obig[pair]: normalized O^T for heads (2*pair, 2*pair+1).
            # Pair-outer loop: each head's S slice owns a full PSUM bank
            # (two start=True groups must never share a bank), and the
            # two-bank S tile double-buffers against the single exp call.
            obig = [obigp.tile([P, N], FP32R, tag=f"obig{pr}", name=f"ob{pr}")
                    for pr in range(2)]
            for pr in range(2):
                for ib in range(NIB):
                    o_ps = [psO.tile([65, IBW], FP32, tag="O",
                                     name=f"o{pr}_{ib}_{s}") for s in range(2)]
                    for jt in range(NJT):
                        s_ps = psS.tile([P, 2 * IBW], FP32, tag="S")
                        for sub in range(2):
                            nc.tensor.matmul(
                                s_ps[:, sub * IBW:(sub + 1) * IBW],
                                hats[("k", pr)][sub * 64:(sub + 1) * 64,
                                                jt * P:(jt + 1) * P],
                                hats[("q", pr)][sub * 64:(sub + 1) * 64,
                                                ib * IBW:(ib + 1) * IBW],
                                start=True, stop=True,
                                tile_position=(64 * sub, 0),
                            )
                        p_sb = ptp.tile([P, 2 * IBW], FP32R, tag="P")
                        nc.scalar.activation(p_sb[:, :], s_ps[:, :],
                                             AF.Exp, scale=0.125)
                        for sub in range(2):
                            h = 2 * pr + sub
                            nc.tensor.matmul(
                                o_ps[sub][:, :],
                                vt[jt][:, h * 65:(h + 1) * 65],
                                p_sb[:, sub * IBW:(sub + 1) * IBW],
                                start=(jt == 0), stop=(jt == NJT - 1),
                            )
                        if pr == 1 and jt % 4 == 2:
                            emit_proj(0, ib * 4 + jt // 4)
                    # normalize: row 64 of each o_ps is the denominator
                    for sub in range(2):
                        rrd = tiny.tile([65, IBW], FP32, tag="rrd")
                        nc.vector.tensor_copy(rrd[64:65, :],
                                              o_ps[sub][64:65, :])
                        nc.vector.reciprocal(rrd[64:65, :], rrd[64:65, :])
                        # partition_broadcast reads the tile's partition 0,
                        # so stage the reciprocal row down via DMA
                        rr0 = tiny.tile([1, IBW], FP32, tag="rr0")
                        nc.sync.dma_start(rr0[:, :], rrd[64:65, :])
                        bc = tiny.tile([64, IBW], FP32, tag="bc")
                        nc.gpsimd.partition_broadcast(bc[:, :], rr0[:, :])
                        if sub == 0:
                            nc.vector.tensor_mul(
                                obig[pr][0:64, ib * IBW:(ib + 1) * IBW],
                                o_ps[sub][0:64, :], bc[:, :])
                        else:
                            onr = onrm.tile([64, IBW], FP32R, tag="onr")
                            nc.vector.tensor_mul(onr[:, :],
                                                 o_ps[sub][0:64, :], bc[:, :])
                            nc.sync.dma_start(
                                obig[pr][64:128, ib * IBW:(ib + 1) * IBW],
                                onr[:, :])

            for tt in range(N // P):
                emit_proj(1, tt)

    nc.compile()
    return nc


# ------------------------------------------------------------------- driver

def _rope_tables():
    half = DH // 2
    inv_freq = 1.0 / (ROPE_BASE ** (np.arange(half, dtype=np.float64) * 2.0
                                    / DH))
    freqs = np.arange(N, dtype=np.float64)[:, None] * inv_freq[None, :]
    cos = np.cos(freqs).T          # (32, N)
    sin = np.sin(freqs).T
    cos64 = np.concatenate([cos, cos], 0)            # (64, N)
    sin64 = np.concatenate([-sin, sin], 0)           # signed for rotate_half
    cos_t = np.ascontiguousarray(
        np.concatenate([cos64, cos64], 0).astype(np.float32))  # (128, N)
    sin_t = np.ascontiguousarray(
        np.concatenate([sin64, sin64], 0).astype(np.float32))
    return cos_t, sin_t


def kernel(input, w_qkv, b_qkv, q_scale, k_scale, w_out, b_out):
    trace = bool(os.environ.get("KERNEL_TRACE"))
    if "l1" not in _cache:
        _cache["l1"] = _build_l1()
    if "l2" not in _cache:
        _cache["l2"] = _build_l2()

    x = np.asarray(input, dtype=np.float32)
    w_qkv = np.asarray(w_qkv, dtype=np.float32)
    b_qkv = np.asarray(b_qkv, dtype=np.float32)
    qs = np.asarray(q_scale, dtype=np.float32)
    ks = np.asarray(k_scale, dtype=np.float32)
    w_out = np.asarray(w_out, dtype=np.float32)
    b_out = np.asarray(b_out, dtype=np.float32)

    wq = w_qkv[:, :DQ] * qs[None, :]
    wk = w_qkv[:, DQ:2 * DQ] * ks[None, :]
    wv = w_qkv[:, 2 * DQ:]
    bq = b_qkv[:DQ] * qs
    bk = b_qkv[DQ:2 * DQ] * ks
    bv = b_qkv[2 * DQ:]

    xT = [np.ascontiguousarray(x[b].T) for b in range(B)]

    def col4(vec256_a, vec256_b):
        # -> (128, 4): [a_mt0 | a_mt1 | b_mt0 | b_mt1]
        return np.ascontiguousarray(np.stack(
            [vec256_a[:P], vec256_a[P:], vec256_b[:P], vec256_b[P:]],
            axis=1).astype(np.float32))

    in1 = []
    for c in range(NCORES):
        b, g = divmod(c, NGROUP)
        sl = slice(g * GF, (g + 1) * GF)
        wcat = np.ascontiguousarray(
            np.concatenate([wq[:, sl], wk[:, sl], wv[:, sl]], axis=1))
        in1.append({
            "xT": xT[b],
            "wcat": wcat,
            "bqk": col4(bq[sl], bk[sl]),
            "invs": np.ascontiguousarray(np.repeat(
                col4(1.0 / np.square(qs[sl]), 1.0 / np.square(ks[sl])),
                32, axis=1)),
        })

    r1 = run_bass_kernel_spmd(_cache["l1"], in1,
                              core_ids=list(range(NCORES)), trace=trace)
    if trace:
        LAST_EXEC_NS["l1"] = r1.exec_time_ns
        LAST_RESULTS["l1"] = r1

    # host: combine partial ssq -> rsqrt factors folded into rope tables
    cos_t, sin_t = _rope_tables()
    tabs = {}
    for b in range(B):
        sq_q = np.zeros(N, np.float64)
        sq_k = np.zeros(N, np.float64)
        for g in range(NGROUP):
            ssq = r1.results[NGROUP * b + g]["ssq"].astype(np.float64)
            sq_q += ssq[0]
            sq_k += ssq[1]
        r_q = (1.0 / np.sqrt(sq_q / DQ + EPS)).astype(np.float32)
        r_k = (1.0 / np.sqrt(sq_k / DQ + EPS)).astype(np.float32)
        tabs[b] = {
            "cosq": np.ascontiguousarray(cos_t * r_q[None, :]),
            "sinq": np.ascontiguousarray(sin_t * r_q[None, :]),
            "cosk": np.ascontiguousarray(cos_t * r_k[None, :]),
            "sink": np.ascontiguousarray(sin_t * r_k[None, :]),
        }

    in2 = []
    for c in range(NCORES):
        b, g = divmod(c, NGROUP)
        sl = slice(g * GF, (g + 1) * GF)
        v = r1.results[c]["v"]                       # (N, 256)
        v65 = np.ones((N, 4 * 65), np.float32)
        for h in range(4):
            v65[:, h * 65:h * 65 + 64] = v[:, h * 64:(h + 1) * 64]
        in2.append({
            "qT": r1.results[c]["qT"],
            "kT": r1.results[c]["kT"],
            "v": np.ascontiguousarray(v65),
            "wout": np.ascontiguousarray(w_out[sl, :]),
            **tabs[b],
        })

    r2 = run_bass_kernel_spmd(_cache["l2"], in2,
                              core_ids=list(range(NCORES)), trace=trace)
    if trace:
        LAST_EXEC_NS["l2"] = r2.exec_time_ns
        LAST_RESULTS["l2"] = r2

    base = (bv.astype(np.float64) @ w_out.astype(np.float64)
            + b_out.astype(np.float64))
    out = np.zeros((B, N, DIN), np.float32)
    for b in range(B):
        acc = np.zeros((N, DIN), np.float64)
        for g in range(NGROUP):
            p = r2.results[NGROUP * b + g]["part"].astype(np.float64)
            acc += p[0]
            acc += p[1]
        out[b] = (acc + base[None, :]).astype(np.float32)
    return out
